# revision 3
# baseline (speedup 1.0000x reference)
"""CrossNetMoE forward on 8 Trainium2 NeuronCores (Bass/Tile).

Math (per layer i, E=4 experts, rank R=64, D=1024):
    v = tanh(V_e @ xl)            [B,E,R]
    c = tanh(C_e @ v_e)           [B,E,R]
    g = softmax(Wg_e . xl)        [B,E]
    u = sum_e (g_e * c_e) @ U_e.T + b      (softmax weights sum to 1)
    xl' = tanh(u * x0 + xl)   (last layer: no tanh)

Strategy: pure data-parallel over batch (2048 rows/core), everything kept in
transposed layout [D, B] on-chip so each layer's matmuls consume the previous
layer's output directly as the PE moving operand; softmax over the 4 experts is
done with tiny auxiliary matmuls (partition reduction / broadcast); the final
`u * x0 + xl` keeps u in PSUM: DVE multiplies x0 in place, then an
identity-weight matmul accumulates xl on top (has_written bits stay set), and
ACT evacuates with the tanh. f32r (TF32-like) matmul throughput is 1 col/cycle.

Host side transposes x once and un-transposes the result; weights are packed
host-side into SBUF-image blobs replicated to all cores.
"""
import json
import os
import sys

sys.path.insert(0, "/opt/trn_rl_repo")

import numpy as np

L, E, D, R = 3, 4, 1024, 64
B = 16384
NCORES = 8
BC = B // NCORES          # 2048 rows per core
N = 256                   # batch columns per group (matmul moving free dim)
G = BC // N               # 8 groups per core
NCH = D // 128            # 8 d-chunks

# per-layer weight blob column offsets (f32 columns, [128, COLS_L])
V_OFF = 0                 # 2 groups x 8 chunks x 128
U_OFF = 2048              # 2 kchunks x 8 mchunks x 128
C_OFF = 4096              # 2 groups x 128
W_OFF = 4352              # 8 chunks x 4
B_OFF = 4384              # 8 chunks x 1
COLS_L = 4392
# common blob
I_OFF = 0                 # identity 128
P_OFF = 128               # 2 groups x 128 (partitions 0..3)
O4_OFF = 384              # ones [4,1]
O14_OFF = 385             # ones [1,4]
COLS_C = 392

_EXEMPT = {"Call"}


def _legalize_json_bytes(raw: bytes) -> bytes:
    """Split multi-wait instructions: walrus allows 1 sync-wait per inst."""
    m = json.loads(raw)
    counter = [0]

    def fix_block(block):
        insts = block.get("instructions")
        if insts is not None:
            out = []
            for inst in insts:
                si = inst.get("sync_info")
                if (
                    si
                    and inst.get("opcode") not in _EXEMPT
                    and len(si.get("on_wait") or []) > 1
                ):
                    for w in si["on_wait"][:-1]:
                        counter[0] += 1
                        out.append(
                            {
                                "name": f"I-waitsplit-{counter[0]}",
                                "opcode": "NoOp",
                                "engine": inst["engine"],
                                "ins": [],
                                "outs": [],
                                "debug": 0,
                                "sync_info": {"on_wait": [w], "on_update": []},
                            }
                        )
                    si["on_wait"] = [si["on_wait"][-1]]
                out.append(inst)
            block["instructions"] = out
        for sub in block.get("blocks") or []:
            fix_block(sub)

    for f in m["functions"]:
        for b in f["blocks"]:
            fix_block(b)
    return json.dumps(m).encode()


def pack_weights(U, V, C, Wg, b):
    """Pack parameters into SBUF-image blobs [128, COLS] (host side)."""
    U, V, C, Wg, b = (np.asarray(a, np.float32) for a in (U, V, C, Wg, b))
    blobs = {}
    p = np.arange(128)
    for l in range(L):
        blob = np.zeros((128, COLS_L), np.float32)
        # Vw: lhsT chunk for vT matmuls: [p=d%128, (g,c,m)]; m -> (e=2g+m//64, r=m%64)
        for g in range(2):
            for c in range(NCH):
                m = np.arange(128)
                # V[l, 2g + m//64, m%64, c*128+p]
                blob[:, V_OFF + (g * 8 + c) * 128 : V_OFF + (g * 8 + c + 1) * 128] = (
                    V[l, 2 * g + m[None, :] // 64, m[None, :] % 64, c * 128 + p[:, None]]
                )
        # Uw: [p=(e,r2)%128 of kchunk, (kc,mc,q)] = U[l, (kc*128+p)//64, mc*128+q, (kc*128+p)%64]
        for kc in range(2):
            for mc in range(NCH):
                q = np.arange(128)
                blob[:, U_OFF + (kc * 8 + mc) * 128 : U_OFF + (kc * 8 + mc + 1) * 128] = (
                    U[l, (kc * 128 + p[:, None]) // 64, mc * 128 + q[None, :], (kc * 128 + p[:, None]) % 64]
                )
        # Cw: blockdiag pairs: [p=(el',r1), (g,j=(el,r2))]
        for g in range(2):
            j = np.arange(128)
            el_p = p[:, None] // 64
            el_j = j[None, :] // 64
            val = C[l, 2 * g + el_j, j[None, :] % 64, p[:, None] % 64]
            blob[:, C_OFF + g * 128 : C_OFF + (g + 1) * 128] = np.where(el_p == el_j, val, 0.0)
        # Ww: [p=d%128, (c,e)]
        for c in range(NCH):
            blob[:, W_OFF + c * 4 : W_OFF + (c + 1) * 4] = Wg[l, :, c * 128 + p]
        # bias
        for c in range(NCH):
            blob[:, B_OFF + c] = b[l, c * 128 + p]
        blobs[f"wl{l}"] = blob
    wc = np.zeros((128, COLS_C), np.float32)
    wc[:, I_OFF : I_OFF + 128] = np.eye(128, dtype=np.float32)
    for g in range(2):
        m = np.arange(128)
        wc[0:4, P_OFF + g * 128 : P_OFF + (g + 1) * 128] = (
            np.arange(4)[:, None] == (2 * g + m[None, :] // 64)
        ).astype(np.float32)
    wc[0:4, O4_OFF] = 1.0
    wc[0:1, O14_OFF : O14_OFF + 4] = 1.0
    blobs["wc"] = wc
    return blobs


def build_nc(bias_nonzero=False):
    import concourse.bass as bass
    import concourse.tile as tile
    from concourse import mybir
    from concourse.tile import add_dep_helper

    f32 = mybir.dt.float32
    f32r = mybir.dt.float32r
    AF = mybir.ActivationFunctionType
    ALU = mybir.AluOpType

    nc = bass.Bass()
    xT = nc.dram_tensor("xT", [D, BC], f32r, kind="ExternalInput")
    wl = [nc.dram_tensor(f"wl{l}", [128, COLS_L], f32r, kind="ExternalInput") for l in range(L)]
    wcd = nc.dram_tensor("wc", [128, COLS_C], f32r, kind="ExternalInput")
    outT = nc.dram_tensor("outT", [D, BC], f32, kind="ExternalOutput")

    # chain matmuls that share a psum tile so scheduler keeps program order
    last_mm = {}

    def mm(key, out, lhsT, rhs, start, stop):
        inst = nc.tensor.matmul(out, lhsT, rhs, start=start, stop=stop, skip_group_check=True)
        if key in last_mm:
            add_dep_helper(inst.ins, last_mm[key].ins, sync=False, reason="psum order")
        last_mm[key] = inst
        return inst

    with tile.TileContext(nc) as tc:
        with (
            tc.tile_pool(name="wpool", bufs=1) as wpool,
            tc.tile_pool(name="xpool", bufs=1) as xpool,
            tc.tile_pool(name="mid", bufs=1) as mid,
            tc.tile_pool(name="pspool", bufs=1, space="PSUM") as pspool,
            nc.allow_low_precision(reason="f32r matmul pipeline (intentional)"),
        ):
            wt = []
            for l in range(L):
                w_l = wpool.tile([128, COLS_L], f32r, name=f"wt{l}")
                nc.sync.dma_start(w_l[:], wl[l][:])
                wt.append(w_l)
            wct = wpool.tile([128, COLS_C], f32r)
            nc.sync.dma_start(wct[:], wcd[:])
            ident = wct[:, I_OFF : I_OFF + 128]

            # dram view for group g: [128, (c, n)] with partition = d%128
            xT_v = xT.rearrange("(c p) b -> p c b", p=128)

            for g in range(G):
                x0t = xpool.tile([128, NCH * N], f32r, tag="x0", bufs=3, name=f"x0_{g}")
                nc.sync.dma_start(x0t[:], xT_v[:, :, g * N : (g + 1) * N])
                xin = x0t
                for l in range(L):
                    kv = f"v{g}_{l}"
                    ks = f"s{g}_{l}"
                    v_ps = pspool.tile([128, 512], f32, tag="v", bufs=1, name=f"vps{g}_{l}")
                    s_ps = pspool.tile([128, 512], f32, tag="sml", bufs=2, name=f"sps{g}_{l}")
                    for c in range(NCH):
                        rhs = xin[:, c * N : (c + 1) * N]
                        mm(kv, v_ps[:, 0:N], wt[l][:, V_OFF + c * 128 : V_OFF + (c + 1) * 128], rhs, start=(c == 0), stop=False)
                        mm(kv, v_ps[:, N : 2 * N], wt[l][:, V_OFF + (8 + c) * 128 : V_OFF + (9 + c) * 128], rhs, start=False, stop=(c == NCH - 1))
                        mm(ks, s_ps[0:4, 0:N], wt[l][:, W_OFF + c * 4 : W_OFF + (c + 1) * 4], rhs, start=(c == 0), stop=(c == NCH - 1))
                    vt = mid.tile([128, 512], f32r, tag="vt", bufs=3, name=f"vt{g}_{l}")
                    nc.scalar.activation(vt[:], v_ps[:], AF.Tanh)
                    e_s = mid.tile([4, N], f32r, tag="es", bufs=3, name=f"es{g}_{l}")
                    nc.scalar.activation(e_s[:], s_ps[0:4, 0:N], AF.Exp)
                    # Z = sum_e exp(s) -> [1, N] (cols N:2N of the small bank)
                    mm(ks, s_ps[0:1, N : 2 * N], wct[0:4, O4_OFF : O4_OFF + 1], e_s[:], start=False, stop=True)
                    rr = mid.tile([1, N], f32r, tag="rr", bufs=3, name=f"rr{g}_{l}")
                    nc.vector.reciprocal(rr[:], s_ps[0:1, N : 2 * N])
                    # r4 = broadcast recip to 4 partitions (reuses cols 0:N)
                    mm(ks, s_ps[0:4, 0:N], wct[0:1, O14_OFF : O14_OFF + 4], rr[:], start=True, stop=True)
                    g4 = mid.tile([4, N], f32r, tag="g4", bufs=3, name=f"g4{g}_{l}")
                    nc.vector.tensor_mul(g4[:], e_s[:].bitcast(f32), s_ps[0:4, 0:N])
                    # c = tanh(blockdiag(C) @ v)
                    kc = f"c{g}_{l}"
                    c_ps = pspool.tile([128, 512], f32, tag="cb", bufs=1, name=f"cps{g}_{l}")
                    mm(kc, c_ps[:, 0:N], wt[l][:, C_OFF : C_OFF + 128], vt[:, 0:N], start=True, stop=True)
                    mm(kc, c_ps[:, N : 2 * N], wt[l][:, C_OFF + 128 : C_OFF + 256], vt[:, N : 2 * N], start=False, stop=True)
                    ct = mid.tile([128, 512], f32r, tag="ct", bufs=3, name=f"ct{g}_{l}")
                    nc.scalar.activation(ct[:], c_ps[:], AF.Tanh)
                    # bg = per-expert gate broadcast over the 64 rank rows
                    kb = f"b{g}_{l}"
                    b_ps = pspool.tile([128, 512], f32, tag="cb", bufs=1, name=f"bps{g}_{l}")
                    mm(kb, b_ps[:, 0:N], wct[0:4, P_OFF : P_OFF + 128], g4[:], start=True, stop=True)
                    mm(kb, b_ps[:, N : 2 * N], wct[0:4, P_OFF + 128 : P_OFF + 256], g4[:], start=False, stop=True)
                    cg = mid.tile([128, 512], f32r, tag="cg", bufs=3, name=f"cg{g}_{l}")
                    nc.vector.tensor_mul(cg[:], ct[:].bitcast(f32), b_ps[:])

                    if l < L - 1:
                        xout = xpool.tile([128, NCH * N], f32r, tag="xl", bufs=4, name=f"xl{g}_{l}")
                    else:
                        xout = xpool.tile([128, NCH * N], f32, tag="osb", bufs=2, name=f"osb{g}")
                    for h in range(2):
                        ku = f"u{g}_{l}_{h}"
                        u_ps = pspool.tile([128, 1024], f32, tag="u", bufs=2, name=f"ups{g}_{l}_{h}")
                        for mc in range(4 * h, 4 * h + 4):
                            col = (mc - 4 * h) * N
                            first_bank = (mc - 4 * h) % 2 == 0
                            for kch in range(2):
                                mm(
                                    ku,
                                    u_ps[:, col : col + N],
                                    wt[l][:, U_OFF + (kch * 8 + mc) * 128 : U_OFF + (kch * 8 + mc + 1) * 128],
                                    cg[:, kch * N : (kch + 1) * N],
                                    start=(first_bank and kch == 0),
                                    stop=(kch == 1),
                                )
                        # u *= x0  (in place in PSUM; optionally +bias first)
                        if bias_nonzero:
                            for mc in range(4 * h, 4 * h + 4):
                                col = (mc - 4 * h) * N
                                nc.vector.scalar_tensor_tensor(
                                    u_ps[:, col : col + N],
                                    u_ps[:, col : col + N],
                                    wt[l][:, B_OFF + mc : B_OFF + mc + 1].bitcast(f32),
                                    x0t[:, mc * N : (mc + 1) * N].bitcast(f32),
                                    ALU.add,
                                    ALU.mult,
                                )
                        else:
                            nc.vector.tensor_mul(
                                u_ps[:], u_ps[:], x0t[:, h * 1024 : (h + 1) * 1024].bitcast(f32)
                            )
                        # u += xl  (identity matmul accumulate; has_written still set)
                        for mc in range(4 * h, 4 * h + 4):
                            col = (mc - 4 * h) * N
                            mm(ku, u_ps[:, col : col + N], ident, xin[:, mc * N : (mc + 1) * N], start=False, stop=True)
                        if l < L - 1:
                            nc.scalar.activation(xout[:, h * 1024 : (h + 1) * 1024], u_ps[:], AF.Tanh)
                        else:
                            nc.scalar.activation(xout[:, h * 1024 : (h + 1) * 1024], u_ps[:], AF.Copy)
                    xin = xout
                outT_v = outT.rearrange("(c p) b -> p c b", p=128)
                nc.sync.dma_start(outT_v[:, :, g * N : (g + 1) * N], xin[:])

    # walrus wait-budget legalization on serialization
    orig = nc.to_json_bytes
    nc.to_json_bytes = lambda: _legalize_json_bytes(orig())
    return nc


_CACHE = {}


def kernel(x, U, V, C, Wg, b):
    x = np.ascontiguousarray(np.asarray(x, np.float32))
    bias_nonzero = bool(np.any(np.asarray(b) != 0))
    key = ("nc", bias_nonzero)
    if key not in _CACHE:
        _CACHE[key] = build_nc(bias_nonzero)
        _CACHE[("blobs", bias_nonzero)] = None
    nc = _CACHE[key]
    blobs = pack_weights(U, V, C, Wg, b)
    xTfull = np.ascontiguousarray(x.T)  # [D, B]
    in_maps = []
    for m in range(NCORES):
        im = {"wc": blobs["wc"]}
        for l in range(L):
            im[f"wl{l}"] = blobs[f"wl{l}"]
        im["xT"] = np.ascontiguousarray(xTfull[:, m * BC : (m + 1) * BC])
        in_maps.append(im)
    from concourse import bass2jax

    results = bass2jax.run_bass_via_pjrt(nc, in_maps, n_cores=NCORES)
    out = np.empty((B, D), np.float32)
    for m in range(NCORES):
        out[m * BC : (m + 1) * BC, :] = results[m]["outT"].T
    return out
